# revision 22
# baseline (speedup 1.0000x reference)
"""GCNConv (SpMM + dense GEMM) Trainium2 Bass kernel, 8-core SPMD.

out = segment_sum(vals[:,None] * x[col], row, N) @ weight

Strategy (CAGNET-style 1D row partition):
  - Each core owns N/8 = 12500 destination rows and all edges targeting them.
  - Host groups a core's edges by (window of 128 dest rows, col-chunk of 25000
    source rows), pads each group to whole 128-edge tiles, and equalizes group
    tile-counts across cores so a single SPMD program serves all 8 cores.
  - The kernel is gather-bandwidth-bound (~80 GB/s/core through the 4 SWDGE
    queues, payload-byte-limited), so x is transported as fp16 (256B rows,
    the minimum dma_gather elem granularity) and all padding descriptors are
    skipped: real edges are front-packed per (window, chunk) group, pad slots
    carry idx -1, one dma_gather per group passes the per-core valid count in
    a register (num_idxs_reg read from the `nvalid` input).
  - Per gather batch (<= GB_TILES whole groups), two batched DVE
    tensor_tensor ops build all scaled one-hot masks at 2x rate in a
    transposed layout: mask[p, r, t] = (r == rloc[p, t]) * val[p, t]
    (iota2 is a materialized stride-1 r-table; per-tile DVE ops are ~4x
    slower on HW than batched ones).  The PE scatter-adds via matmul with
    the strided mask slice: z^T[f, r] += G[e, f]^T @ mask[e, :, k].
  - Final per-window GEMM out[r, o] = z^T[f, r]^T @ weight[f, o] in fp32.

No collectives needed: row partition makes each core's output disjoint.
fp16 transport keeps rel err ~3e-4 (PSUM accumulates fp32); tolerance 2e-2.
"""

import numpy as np

N_NODES = 100000
F = 128
P = 128
NCORES = 8
ROWS_PER_CORE = N_NODES // NCORES          # 12500
NWIN = (ROWS_PER_CORE + P - 1) // P        # 98
NCHUNK = 4
CHUNK = N_NODES // NCHUNK                  # 25000
WG = 5                                     # windows per PSUM group
PSUM_BUFS = WG + 1                         # WG live + 1 pipeline
USE_BF16 = True                            # fp16 x-table + gathered G + onehot
GATHER_ONLY = False                        # bench: skip compute, gathers only
SKIP_DVE = False                           # bench: matmul rhs=iota (no onehots)
SKIP_PE = False                            # bench: no matmuls
NQUEUES = 4                                # parallel SWDGE queues for gathers
GB_TILES = 64                              # tiles per gather sub-batch
G_BUFS = 6                                 # gather dst double-buffering
SINGLE_PACKET = False
SBUF_SRC = False                           # probe: gather from SBUF-staged
                                           # chunk tokens (timing only)
NSTRIPE = (CHUNK + P - 1) // P             # 196 token stripes per chunk

SAFE_BATCHES = 6                           # buffer-warmup batches keep 0-pads
                                           # (stale-SBUF NaN guard)

_compiled_cache = {}


def _schedule(T):
    """Shared host/device schedule: batches of whole (w, c) groups.

    Returns a list of batches (wgi, ci, tb, T_sub, groups) where groups is
    a list of (w, kbase, Tg), tb the batch's global tile base, and kbase
    the group's tile offset within the batch.  Iteration order matches the
    packed-tile order: wgroup -> chunk -> window.
    """
    GB = GB_TILES if USE_BF16 else 44
    NWG = (NWIN + WG - 1) // WG
    batches = []
    t0 = 0
    for wgi in range(NWG):
        wins = [w for w in range(wgi * WG, min((wgi + 1) * WG, NWIN))]
        for ci in range(NCHUNK):
            cur, cur_tiles = [], 0
            for w in wins:
                Tg = int(T[w, ci])
                if Tg == 0:
                    continue
                assert Tg <= GB
                if cur and cur_tiles + Tg > GB:
                    batches.append((wgi, ci, t0, cur_tiles, cur))
                    t0 += cur_tiles
                    cur, cur_tiles = [], 0
                cur.append((w, cur_tiles, Tg))
                cur_tiles += Tg
            if cur:
                batches.append((wgi, ci, t0, cur_tiles, cur))
                t0 += cur_tiles
    return batches


# ---------------------------------------------------------------- host prep

def _prep_inputs(row, col, vals, x, weight):
    """Partition + sort + pad edges; build per-core device arrays.

    Returns (in_maps, T) where T is the [NWIN, NCHUNK] tile-count table
    (identical across cores) and in_maps the per-core input dicts.
    """
    row = np.asarray(row).astype(np.int64)
    col = np.asarray(col).astype(np.int64)
    vals = np.asarray(vals).astype(np.float32)
    x = np.ascontiguousarray(np.asarray(x), dtype=np.float32)
    weight = np.ascontiguousarray(np.asarray(weight), dtype=np.float32)

    core = row // ROWS_PER_CORE
    rl = row % ROWS_PER_CORE
    w = rl // P
    rloc = (rl % P).astype(np.float32)
    c = col // CHUNK
    cloc = (col % CHUNK).astype(np.int64)

    # group id per edge within its core: ordered by (wgroup, chunk, window)
    wg = w // WG
    gid = (wg * NCHUNK + c) * WG + (w % WG)          # dense group order
    NGROUPS = ((NWIN + WG - 1) // WG) * NCHUNK * WG  # includes unused tail slots

    flat = core * NGROUPS + gid
    cnt = np.bincount(flat, minlength=NCORES * NGROUPS).reshape(NCORES, NGROUPS)

    # tiles per group: max over cores, ceil to 128; force >=1 tile for chunk 0
    T_g = (cnt.max(axis=0) + P - 1) // P             # [NGROUPS]
    # map group order index -> (w, c) to force chunk-0 min tiles
    order_w = np.zeros(NGROUPS, dtype=np.int64)
    order_c = np.zeros(NGROUPS, dtype=np.int64)
    valid = np.zeros(NGROUPS, dtype=bool)
    for wgi in range((NWIN + WG - 1) // WG):
        for ci in range(NCHUNK):
            for wj in range(WG):
                wi = wgi * WG + wj
                g = (wgi * NCHUNK + ci) * WG + wj
                order_w[g] = wi
                order_c[g] = ci
                valid[g] = wi < NWIN
    T_g[(order_c == 0) & valid & (T_g == 0)] = 1
    T_g[~valid] = 0

    group_tile_base = np.zeros(NGROUPS, dtype=np.int64)
    np.cumsum(T_g[:-1], out=group_tile_base[1:])
    T_tot = int(T_g.sum())

    # per-core padded edge arrays
    # secondary sort by col within each (core, group) for HBM bank locality
    edge_order = np.lexsort((cloc, flat))
    sorted_flat = flat[edge_order]
    # rank within (core, group)
    starts = np.searchsorted(sorted_flat, np.arange(NCORES * NGROUPS))
    rank = np.arange(len(row)) - starts[sorted_flat]
    pos = group_tile_base[sorted_flat % NGROUPS] * P + rank  # position in padded list

    E_pad = T_tot * P
    cols_pad = np.full((NCORES, E_pad), -1, dtype=np.int16)
    vals_pad = np.zeros((NCORES, E_pad), dtype=np.float32)
    rloc_pad = np.zeros((NCORES, E_pad), dtype=np.float32)
    ci_pad = sorted_flat // NGROUPS
    cols_pad[ci_pad, pos] = cloc[edge_order]
    vals_pad[ci_pad, pos] = vals[edge_order]
    rloc_pad[ci_pad, pos] = rloc[edge_order]

    # trailing -1 pad slots are skipped by the gather DMA (num_idxs_reg).
    # First SAFE_BATCHES batches keep 0-pads so gather buffers never expose
    # stale (possibly NaN) SBUF; all gathers keep >=1 valid index.
    T_mat = np.zeros((NWIN, NCHUNK), dtype=np.int64)
    T_mat[order_w[valid], order_c[valid]] = T_g[valid]
    batches = _schedule(T_mat)
    n_groups = sum(len(b[4]) for b in batches)
    nvalid = np.zeros((NCORES, n_groups), dtype=np.int32)
    g_idx = 0
    for bi, (wgi, ci2, tb, T_sub, groups) in enumerate(batches):
        for (w2, kbase, Tg) in groups:
            s0 = (tb + kbase) * P
            s1 = s0 + Tg * P
            assert group_tile_base[(wgi * NCHUNK + ci2) * WG + (w2 % WG)] \
                == tb + kbase
            if bi < SAFE_BATCHES:
                np.maximum(cols_pad[:, s0:s1], 0, out=cols_pad[:, s0:s1])
            else:
                first = cols_pad[:, s0]
                first[first < 0] = 0
                cols_pad[:, s0] = first
            nvalid[:, g_idx] = (cols_pad[:, s0:s1] >= 0).sum(axis=1)
            g_idx += 1

    iota = np.tile(np.arange(P, dtype=np.float32), (P, 1))
    # iota2[p, r*GB + t] = r  (transposed-mask layout, stride GB along t)
    GB = GB_TILES if USE_BF16 else 44
    iota2 = np.repeat(np.arange(P, dtype=np.float32), GB).reshape(
        1, P, GB).repeat(P, 0)
    if USE_BF16:
        x = x.astype(np.float16)
        iota = iota.astype(np.float16)
        iota2 = iota2.astype(np.float16)
        vals_pad = vals_pad.astype(np.float16)
        rloc_pad = rloc_pad.astype(np.float16)

    xtok = None
    if SBUF_SRC:
        xc = np.zeros((NSTRIPE * P, F), dtype=x.dtype)
        xc[:CHUNK] = x[:CHUNK]
        xtok = np.ascontiguousarray(
            xc.reshape(NSTRIPE, P, F).transpose(1, 0, 2).reshape(P, NSTRIPE * F))

    in_maps = []
    for i in range(NCORES):
        # packed [P, T_tot]: edge t*128+p at [p, t]
        v_pk = np.ascontiguousarray(vals_pad[i].reshape(T_tot, P).T)
        r_pk = np.ascontiguousarray(rloc_pad[i].reshape(T_tot, P).T)
        # idx16 wrapped: edge i at [i%16, i//16], replicated to 128 partitions
        idx_w = np.ascontiguousarray(cols_pad[i].reshape(E_pad // 16, 16).T)
        idx16 = np.ascontiguousarray(np.tile(idx_w, (8, 1)))
        m = dict(
            x=x, cols16=idx16, vals=v_pk, rloc=r_pk, iota=iota, iota2=iota2,
            weight=weight, nvalid=nvalid[i:i + 1],
        )
        if SBUF_SRC:
            m["xtok"] = xtok
        in_maps.append(m)

    return in_maps, T_mat


# ---------------------------------------------------------------- device program

def _build_program(T):
    import concourse.mybir as mybir
    import concourse.tile as tile
    from concourse import bacc
    from concourse import library_config

    f32 = mybir.dt.float32
    gdt = mybir.dt.float16 if USE_BF16 else f32
    T_tot = int(T.sum())
    E_pad = T_tot * P

    nc = bacc.Bacc("TRN2", target_bir_lowering=False, debug=True,
                   num_swdge_queues=NQUEUES)

    x_d = nc.dram_tensor("x", [N_NODES, F], gdt, kind="ExternalInput")
    cols_d = nc.dram_tensor("cols16", [P, E_pad // 16], mybir.dt.int16,
                            kind="ExternalInput")
    vals_d = nc.dram_tensor("vals", [P, T_tot], gdt, kind="ExternalInput")
    rloc_d = nc.dram_tensor("rloc", [P, T_tot], gdt, kind="ExternalInput")
    iota_d = nc.dram_tensor("iota", [P, P], gdt, kind="ExternalInput")
    iota2_d = nc.dram_tensor("iota2", [P, P * (GB_TILES if USE_BF16 else 44)],
                             gdt, kind="ExternalInput")
    w_d = nc.dram_tensor("weight", [F, F], f32, kind="ExternalInput")
    out_d = nc.dram_tensor("out", [ROWS_PER_CORE, F], f32, kind="ExternalOutput")
    if SBUF_SRC:
        xtok_d = nc.dram_tensor("xtok", [P, NSTRIPE * F], gdt,
                                kind="ExternalInput")
    batches = _schedule(T)
    n_groups = sum(len(b[4]) for b in batches)
    nvalid_d = nc.dram_tensor("nvalid", [1, n_groups], mybir.dt.int32,
                              kind="ExternalInput")

    # per-window (first, last) tile ownership for PSUM start/stop flags
    nz_chunks = [[c for c in range(NCHUNK) if T[w, c] > 0] for w in range(NWIN)]

    # SBUF-driven sizing: fp32 tiles are 2x, shrink batches to fit
    GB = GB_TILES if USE_BF16 else 44
    GBUF = G_BUFS if USE_BF16 else 3
    assert not SBUF_SRC or GATHER_ONLY, "SBUF_SRC is a timing probe"

    NWG = (NWIN + WG - 1) // WG

    with tile.TileContext(nc) as tc:
        with (
            tc.tile_pool(name="const", bufs=1) as const_pool,
            tc.tile_pool(name="z", bufs=2) as z_pool,
            tc.tile_pool(name="idx", bufs=GBUF) as idx_pool,
            tc.tile_pool(name="meta", bufs=GBUF) as meta_pool,
            tc.tile_pool(name="g", bufs=GBUF) as g_pool,
            tc.tile_pool(name="oh", bufs=2) as oh_pool,
            tc.tile_pool(name="ostage", bufs=4) as ostage_pool,
            tc.tile_pool(name="psum", bufs=PSUM_BUFS, space="PSUM") as psum_pool,
            tc.tile_pool(name="opsum", bufs=2, space="PSUM") as opsum_pool,
        ):
            iota_s = const_pool.tile([P, P], gdt)
            nc.sync.dma_start(out=iota_s[:], in_=iota_d[:])
            iota2_s = const_pool.tile([P, P, GB], gdt)
            nc.sync.dma_start(out=iota2_s[:], in_=iota2_d[:])
            w_s = const_pool.tile([F, F], f32)
            nc.sync.dma_start(out=w_s[:], in_=w_d[:])
            if SBUF_SRC:
                xtok_s = const_pool.tile([P, NSTRIPE * F], gdt)
                nc.sync.dma_start(out=xtok_s[:], in_=xtok_d[:])
            nvalid_s = const_pool.tile([1, n_groups], mybir.dt.int32)
            nc.sync.dma_start(out=nvalid_s[:], in_=nvalid_d[:])

            n_gathers = 0
            g_idx = 0
            qload = [0] * NQUEUES
            psum_tiles = {}
            bi = 0
            nb = len(batches)
            with nc.gpsimd.register(name="nv") as nv_reg:
                while bi < nb:
                    wgi = batches[bi][0]
                    wins = [w for w in range(wgi * WG,
                                             min((wgi + 1) * WG, NWIN))]
                    while bi < nb and batches[bi][0] == wgi:
                        _, ci, tb, T_sub, groups = batches[bi]
                        bi += 1
                        idx_s = idx_pool.tile([P, T_sub * 8], mybir.dt.int16)
                        nc.sync.dma_start(
                            out=idx_s[:],
                            in_=cols_d[:, tb * 8:(tb + T_sub) * 8])
                        v_s = meta_pool.tile([P, T_sub], gdt, tag="v")
                        nc.sync.dma_start(out=v_s[:],
                                          in_=vals_d[:, tb:tb + T_sub])
                        r_s = meta_pool.tile([P, T_sub], gdt, tag="r")
                        nc.sync.dma_start(out=r_s[:],
                                          in_=rloc_d[:, tb:tb + T_sub])

                        g_s = g_pool.tile([P, GB, F], gdt)
                        for (w, kbase, Tg) in groups:
                            nc.gpsimd.load(nv_reg,
                                           nvalid_s[0:1, g_idx:g_idx + 1])
                            g_idx += 1
                            q = min(range(NQUEUES), key=lambda i: qload[i])
                            qload[q] += Tg
                            nc.gpsimd.dma_gather(
                                g_s[:, kbase:kbase + Tg, :],
                                x_d[ci * CHUNK:(ci + 1) * CHUNK, :],
                                idx_s[:, kbase * 8:(kbase + Tg) * 8],
                                Tg * P,
                                nv_reg,
                                F,
                                single_packet=SINGLE_PACKET,
                                queue_num=q,
                            )
                            n_gathers += 1

                        if GATHER_ONLY:
                            continue
                        if not SKIP_DVE:
                            # batched transposed scaled-onehot, 2 DVE ops
                            # per batch: mask[p, r, t] =
                            #   (r == rloc[p, t]) * val[p, t]
                            mask_b = oh_pool.tile([P, P, GB], gdt,
                                                  name="mask", tag="mask")
                            nc.vector.tensor_tensor(
                                out=mask_b[:, :, :T_sub],
                                in0=iota2_s[:, :, :T_sub],
                                in1=r_s[:].unsqueeze(1)
                                    .broadcast_to([P, P, T_sub]),
                                op=mybir.AluOpType.is_equal,
                            )
                            nc.vector.tensor_tensor(
                                out=mask_b[:, :, :T_sub],
                                in0=mask_b[:, :, :T_sub],
                                in1=v_s[:].unsqueeze(1)
                                    .broadcast_to([P, P, T_sub]),
                                op=mybir.AluOpType.mult,
                            )
                        if SKIP_PE:
                            continue
                        for (w, kbase, Tg) in groups:
                            for j in range(Tg):
                                k = kbase + j
                                first = (ci == nz_chunks[w][0]) and (j == 0)
                                last = (ci == nz_chunks[w][-1]) and \
                                    (j == Tg - 1)
                                if first:
                                    psum_tiles[w] = psum_pool.tile(
                                        [P, P], f32, name="zt", tag="zt")
                                nc.tensor.matmul(
                                    out=psum_tiles[w][:],
                                    lhsT=g_s[:, k, :],
                                    rhs=mask_b[:, :, k] if not SKIP_DVE
                                    else iota_s[:],
                                    start=first,
                                    stop=last,
                                )
                    # wgroup epilogue, interleaved with later wgroups'
                    # gathers: PSUM z -> SBUF staging, then the per-window
                    # output GEMM out[r, :] = z^T.T @ W and its store
                    z_t = z_pool.tile([P, WG * P], f32)
                    if GATHER_ONLY or SKIP_PE:
                        nc.vector.memset(z_t[:], 0.0)
                    else:
                        for wj, w in enumerate(wins):
                            nc.scalar.copy(out=z_t[:, wj * P:(wj + 1) * P],
                                           in_=psum_tiles.pop(w)[:])
                    for wj, w in enumerate(wins):
                        rows = min(P, ROWS_PER_CORE - w * P)
                        o_psum = opsum_pool.tile([P, F], f32)
                        nc.tensor.matmul(
                            out=o_psum[:rows, :],
                            lhsT=z_t[:, wj * P:wj * P + rows],
                            rhs=w_s[:],
                            start=True,
                            stop=True,
                        )
                        o_stage = ostage_pool.tile([P, F], f32)
                        nc.scalar.copy(out=o_stage[:rows, :],
                                       in_=o_psum[:rows, :])
                        nc.sync.dma_start(out=out_d[w * P:w * P + rows, :],
                                          in_=o_stage[:rows, :])

    nc.compile()
    return nc


# ---------------------------------------------------------------- entry point

def kernel(row, col, vals, x, weight):
    from concourse.bass_utils import run_bass_kernel_spmd

    in_maps, T = _prep_inputs(row, col, vals, x, weight)

    key = T.tobytes()
    nc = _compiled_cache.get(key)
    if nc is None:
        nc = _build_program(T)
        _compiled_cache[key] = nc

    res = run_bass_kernel_spmd(nc, in_maps, list(range(NCORES)))
    out = np.concatenate([res.results[i]["out"] for i in range(NCORES)], axis=0)
    return out

